# revision 52
# baseline (speedup 1.0000x reference)
"""Self-contained Trainium2 kernel for nn_Block (dense transformer block),
8-way batch-parallel across NeuronCores.

fp8 version.  All matmul operands are fp8 (e4m3 except w_proj in e5m2);
contraction-pair packing via MatmulPerfMode.DoubleRow (two K=128 chunks
per instruction) on every weight-stationary matmul and on P@V.  Scores
stay plain fp8 (K=64 contraction can't pair).  fp32 accumulation in
PSUM throughout; residuals, LN stats and softmax denominators fp32.

Scale plumbing (zero extra instructions):
  - LN outputs are scaled x16 (folded into rstd via sqrt((var+eps)/256))
    and ln biases are pre-scaled x16 on the host.
  - w_qkv/w_fc1/w_fc2 are pre-scaled by a power of two (absmax -> ~224)
    on the host; the inverse rides existing evacuation scale slots
    (tensor_scalar mult / gelu input scale).
  - softmax: the ones column in v is 1/64 so the P@V denominator row is
    sum(exp)/64; o^T = pv / bcast(denom/64) = 64*o lands in e4m3 range,
    and w_proj is pre-divided by 64 (e5m2) so proj PSUM is true-scale.

Layouts: token-major LN with bn_stats (x lives in one [128, 8, 768]
tile); PE-transposes to feature-major run as groups of four bf16
[128,128] blocks into one [128,512] PSUM tile (a single accumulation
group with disjoint writes - the bank's lazy zero makes them plain
stores), evacuated by one instruction that applies the LN affine and
the fp8 cast (alternating ACT/DVE).  hT/h2T/oT live as single
[128, 6, 1024] tiles and gT as [128, 24, 1024] so DoubleRow rhs/lhsT
pairs are plain slices.  v is token-major [128, 2, 12, 128] per kc-pair
(64 v dims, a 1/64 column emitting denom/64 in PSUM row 64, padding for
the dual-fp8 ldweights 16B alignment rules); q^T,k^T are feature-major
[128, 1024] per block; exp(S^T) is written by ACT directly to fp8 into
kc-paired, hp-parity double-buffered [128, 2, 1024] tiles; P@V
DoubleRow contracts 256 k-tokens per instruction.  The attention loop
is software-pipelined: PV(hp-1) + its softmax normalization run between
the first two and the remaining six kc score/exp rounds of pair hp, so
ACT (the attention bottleneck: 96 exps of [128,1024]) never starves.
Softmax normalization: ACT exp(-ln(denom/64)) on the [1, N] denominator
row, DMA broadcast via a DRAM bounce, DVE multiply into fp8 oT (x64).
proj/MLP weight DMAs are issued mid-attention to hide their transfers.
The fc2 output transposes back token-major via grouped PE transposes
with the residual added in place; each token's output DMA issues as
soon as its last channel block lands.

Measured (neuron-profile, 8 cores): ~350-370us vs 461us baseline; PE
~230us busy (DoubleRow K=256 at 1 col/cycle = 2x bf16 FLOPs; K=32 DR
sub-tiles measured SLOWER - don't fold scores), ACT ~185us (exp 101us
is the attention floor), DVE ~114us.  Run-to-run clock variance ~10%.
"""

import numpy as _np
import ml_dtypes as _mld

import concourse.bass as bass
import concourse.mybir as mybir
from concourse.masks import make_identity

AF = mybir.ActivationFunctionType
ALU = mybir.AluOpType
DR = mybir.MatmulPerfMode.DoubleRow
FP32 = mybir.dt.float32
BF16 = mybir.dt.bfloat16
E4 = mybir.dt.float8e4
E5 = mybir.dt.float8e5

N, C, H, HD, HID = 1024, 768, 12, 64, 4 * 768
P = 128
TOK = N // P        # 8 token chunks
CT = C // P         # 6 channel chunks
CJ = CT // 2        # 3 channel DoubleRow pairs
HIDT = HID // P     # 24 hidden chunks
HJ = HIDT // 2      # 12 hidden DoubleRow pairs
KCJ = TOK // 2      # 4 k-token DoubleRow pairs
EPS = 1e-5
SCALE = HD ** (-0.5)
SA = 16.0           # LN activation pre-scale (folded into rstd)
OSC = 64.0          # softmax output pre-scale (ones col = 1/OSC)


def build(nc: bass.Bass, tc, s_qkv, s_fc1, s_fc2,
          with_b_proj=True, with_b_fc2=True):
    ctx_lp = nc.allow_low_precision(
        reason="fp8 matmul operands, fp32 accum; validated vs fp32 ref"
    )
    ctx_lp.__enter__()
    x = nc.dram_tensor("x", [N, C], FP32, kind="ExternalInput").ap()
    ln1_g = nc.dram_tensor("ln1_g", [C], FP32, kind="ExternalInput").ap()
    ln1_b = nc.dram_tensor("ln1_b", [C], FP32, kind="ExternalInput").ap()
    w_qkv = nc.dram_tensor("w_qkv", [C, 3 * C], E4, kind="ExternalInput").ap()
    w_proj = nc.dram_tensor("w_proj", [C, C], E5, kind="ExternalInput").ap()
    b_proj = nc.dram_tensor("b_proj", [C], FP32, kind="ExternalInput").ap()
    ln2_g = nc.dram_tensor("ln2_g", [C], FP32, kind="ExternalInput").ap()
    ln2_b = nc.dram_tensor("ln2_b", [C], FP32, kind="ExternalInput").ap()
    w_fc1 = nc.dram_tensor("w_fc1", [C, HID], E4, kind="ExternalInput").ap()
    b_fc1 = nc.dram_tensor("b_fc1", [HID], FP32, kind="ExternalInput").ap()
    w_fc2 = nc.dram_tensor("w_fc2", [HID, C], E4, kind="ExternalInput").ap()
    b_fc2 = nc.dram_tensor("b_fc2", [C], FP32, kind="ExternalInput").ap()
    out = nc.dram_tensor("out", [N, C], FP32, kind="ExternalOutput").ap()

    iq = 1.0 / (SA * s_qkv)   # q/k/v evacuation scale
    ig = 1.0 / (SA * s_fc1)   # gelu input scale
    iy = 1.0 / s_fc2          # fc2 evacuation scale

    with (
        tc.tile_pool(name="singles", bufs=1) as singles,
        tc.tile_pool(name="xpool", bufs=1) as xpool,
        tc.tile_pool(name="wpool", bufs=1) as wpool,
        tc.tile_pool(name="temps", bufs=4) as temps,
        tc.tile_pool(name="stats", bufs=4) as stats,
    ):
        eps_t = singles.tile([P, 1], FP32, tag="eps", name="eps")
        nc.vector.memset(eps_t, EPS / (SA * SA))
        identB = singles.tile([P, P], BF16, tag="identB", name="identB")
        make_identity(nc, identB)

        def col_load(vec_ap, n_ch, tag):
            """[n_ch*128] DRAM vector -> [128, n_ch] SBUF per-partition."""
            t = singles.tile([P, n_ch], FP32, tag=tag, name=tag)
            nc.sync.dma_start(out=t, in_=vec_ap.rearrange("(c p) -> p c", p=P))
            return t

        def bcast_load(vec_ap, tag):
            """[768] DRAM vector -> [128, 768] broadcast across partitions."""
            t = singles.tile([P, C], FP32, tag=tag, name=tag)
            src = bass.AP(
                tensor=vec_ap.tensor,
                offset=vec_ap.offset,
                ap=[[0, P], *vec_ap.ap],
            )
            nc.sync.dma_start(out=t, in_=src)
            return t

        g1c = col_load(ln1_g, CT, "g1c")
        b1c = col_load(ln1_b, CT, "b1c")  # pre-scaled x16 on host
        g2c = col_load(ln2_g, CT, "g2c")
        b2c = col_load(ln2_b, CT, "b2c")  # pre-scaled x16 on host
        bf1c = col_load(b_fc1, HIDT, "bf1c")
        bp_b = bcast_load(b_proj, "bp_b") if with_b_proj else None
        bf2_b = bcast_load(b_fc2, "bf2_b") if with_b_fc2 else None

        # --- weights: DoubleRow-paired SBUF tiles, loaded up front ------
        def dr_load(src2d, r0, c0, m, dtype, tag):
            """rows [r0, r0+256), cols [c0, c0+m) -> [128, 2, m] tile."""
            t = wpool.tile([P, 2, m], dtype, tag=tag, name=tag)
            nc.sync.dma_start(
                out=t,
                in_=src2d[r0 : r0 + 2 * P, c0 : c0 + m].rearrange(
                    "(i p) m -> p i m", i=2
                ),
            )
            return t

        # --- load x first (critical path), then qkv weights; proj/MLP
        # weights are loaded later so their DMAs ride the attention phase
        xtall = xpool.tile([P, TOK, C], FP32, tag="xtall", name="xtall")
        for m in range(TOK):
            nc.sync.dma_start(
                out=xtall[:, m, :], in_=x[m * P : (m + 1) * P, :]
            )
        wq = [dr_load(w_qkv, 2 * j * P, 0, 2 * C, E4, f"wq{j}") for j in range(CJ)]
        wv = [dr_load(w_qkv, 2 * j * P, 2 * C, C, E4, f"wv{j}") for j in range(CJ)]
        wp, wf1, wf2 = [], [], []  # loaded mid-attention (see hp loop)

        def ln_normalize(src_tile):
            """token-major [128, 768] fp32 -> bf16  16*(x-mu)/sigma."""
            st = stats.tile([P, 3, 6], FP32, tag="bnst", name="bnst")
            src3 = src_tile.rearrange("p (s d) -> p s d", s=3)
            for s in range(3):
                nc.vector.bn_stats(out=st[:, s, :], in_=src3[:, s, :])
            mv = stats.tile([P, 2], FP32, tag="bnmv", name="bnmv")
            nc.vector.bn_aggr(out=mv, in_=st)
            rstd = stats.tile([P, 1], FP32, tag="bnrstd", name="bnrstd")
            nc.scalar.activation(
                out=rstd, in_=mv[:, 1:2], func=AF.Sqrt, bias=eps_t,
                scale=1.0 / (SA * SA),
            )
            nc.vector.reciprocal(out=rstd, in_=rstd)  # = 16/sigma
            hn = temps.tile([P, C], BF16, tag="hn", name="hn")
            nc.vector.tensor_scalar(
                out=hn, in0=src_tile, scalar1=mv[:, 0:1], scalar2=rstd,
                op0=ALU.subtract, op1=ALU.mult,
            )
            return hn

        def ln_to_feature_major(gcol, bcol, h8, fold_bias):
            """LN each token tile, PE-transpose in groups of 4 blocks into
            one [128, 512] bf16 PSUM tile (single accumulation group with
            disjoint writes), evacuate with the LN affine + fp8 cast."""
            with tc.tile_pool(name="psT", bufs=1, space="PSUM") as pstp:
                for half in range(2):
                    hns = []
                    for mm4 in range(4):
                        m = half * 4 + mm4
                        hns.append(ln_normalize(xtall[:, m, :]))
                    for c in range(CT):
                        pst = pstp.tile(
                            [P, 512], BF16, tag=f"pst{c}", name=f"pst{c}"
                        )
                        for mm4 in range(4):
                            nc.tensor.matmul(
                                pst[:, mm4 * P : (mm4 + 1) * P],
                                lhsT=hns[mm4][:, c * P : (c + 1) * P],
                                rhs=identB,
                                is_transpose=True,
                                start=(mm4 == 0), stop=(mm4 == 3),
                                skip_group_check=True,
                            )
                        dst = h8[:, c, half * 512 : (half + 1) * 512]
                        if c % 2 == 0:
                            nc.scalar.activation(
                                out=dst, in_=pst, func=AF.Identity,
                                scale=gcol[:, c : c + 1],
                                bias=bcol[:, c : c + 1],
                            )
                        else:
                            nc.vector.tensor_scalar(
                                out=dst, in0=pst,
                                scalar1=gcol[:, c : c + 1],
                                scalar2=bcol[:, c : c + 1],
                                op0=ALU.mult, op1=ALU.add,
                            )
                    if fold_bias is not None:
                        for mm4 in range(4):
                            m = half * 4 + mm4
                            nc.vector.tensor_tensor(
                                out=xtall[:, m, :], in0=xtall[:, m, :],
                                in1=fold_bias, op=ALU.add,
                            )

        with tc.tile_pool(name="hTpool", bufs=1) as hTpool:
            hT8 = hTpool.tile([P, CT, N], E4, tag="hT8", name="hT8")
            ln_to_feature_major(g1c, b1c, hT8, bp_b)

            with tc.tile_pool(name="qkTpool", bufs=1) as qkTpool:
                qkT = [
                    qkTpool.tile([P, N], E4, tag=f"qkT{i}", name=f"qkT{i}")
                    for i in range(2 * CT)
                ]
                with tc.tile_pool(name="vxpool", bufs=1) as vxpool:
                    # per head: 64 v columns, a 1/64 column emitting
                    # denom/64 in PSUM row 64, then padding to 128 —
                    # dual-fp8 ldweights needs 16B-aligned strides/bases
                    vx = [
                        vxpool.tile(
                            [P, 2, H, 2 * HD], E4, tag=f"vx{jc}", name=f"vx{jc}"
                        )
                        for jc in range(KCJ)
                    ]
                    # v ones columns (v itself is produced inside the
                    # attention phase, riding pair 0's exp shadow)
                    for jc in range(KCJ):
                        for i in range(2):
                            nc.vector.memset(
                                vx[jc][:, i, :, HD : 2 * HD], 1.0 / OSC
                            )

                    with tc.tile_pool(name="oTpool", bufs=1) as oTpool:
                      oT8 = oTpool.tile(
                          [P, CT, N], E4, tag="oT8", name="oT8"
                      )
                      with (
                        tc.tile_pool(name="psSc", bufs=2, space="PSUM") as psSc,
                        tc.tile_pool(name="psO", bufs=2, space="PSUM") as psO,
                        tc.tile_pool(name="expp", bufs=1) as expp,
                        tc.tile_pool(name="attn_s", bufs=4) as attn_s,
                        tc.tile_pool(name="attn_b", bufs=3) as attn_b,
                        tc.tile_pool(name="rsd", bufs=4, space="DRAM") as rsd,
                      ):
                        # double-buffered over head pairs (hp parity) so
                        # PV(hp-1) reads while scores(hp) write
                        expS = [
                            {
                                (sub, jc): expp.tile(
                                    [P, 2, N], E4,
                                    tag=f"expS{par}_{sub}_{jc}",
                                    name=f"expS{par}_{sub}_{jc}",
                                )
                                for sub in range(2)
                                for jc in range(KCJ)
                            }
                            for par in range(2)
                        ]

                        def make_qkT(i):
                            ps = psO.tile([P, N], FP32, tag="ops", name="ops")
                            for h in range(2):
                                for j in range(CJ):
                                    nc.tensor.matmul(
                                        ps[:, h * 512 : (h + 1) * 512],
                                        lhsT=wq[j][:, :, i * P : (i + 1) * P],
                                        rhs=hT8[:, 2 * j : 2 * j + 2,
                                                h * 512 : (h + 1) * 512],
                                        start=(j == 0),
                                        stop=(j == CJ - 1),
                                        perf_mode=DR,
                                    )
                            nc.vector.tensor_scalar(
                                out=qkT[i], in0=ps, scalar1=iq, scalar2=None,
                                op0=ALU.mult,
                            )

                        def scores_exp(hp, kc):
                            qt = qkT[hp]
                            kt = qkT[CT + hp]
                            for sub in range(2):
                                sp = psSc.tile(
                                    [P, N], FP32, tag="scps", name="scps"
                                )
                                rows = slice(sub * HD, (sub + 1) * HD)
                                for qh in range(2):
                                    nc.tensor.matmul(
                                        sp[:, qh * 512 : (qh + 1) * 512],
                                        lhsT=kt[rows, kc * P : (kc + 1) * P],
                                        rhs=qt[rows,
                                               qh * 512 : (qh + 1) * 512],
                                        start=True,
                                        stop=True,
                                    )
                                nc.scalar.activation(
                                    out=expS[hp % 2][sub, kc // 2][
                                        :, kc % 2, :
                                    ],
                                    in_=sp, func=AF.Exp, scale=SCALE,
                                )

                        pv_tiles = {}

                        def pv_chunk(hp, sub, qh):
                            # P@V quarter (DoubleRow over kc pairs); the
                            # 1/64 column puts denom/64 in PSUM row 64
                            head = 2 * hp + sub
                            if qh == 0:
                                pv_tiles[sub] = psO.tile(
                                    [P, N], FP32, tag="ops", name="ops"
                                )
                            ops = pv_tiles[sub]
                            qs = slice(qh * 512, (qh + 1) * 512)
                            for jc in range(KCJ):
                                nc.tensor.matmul(
                                    ops[:, qs],
                                    lhsT=vx[jc][:, :, head, :],
                                    rhs=expS[hp % 2][sub, jc][:, :, qs],
                                    start=(jc == 0),
                                    stop=(jc == KCJ - 1),
                                    perf_mode=DR,
                                )

                        def denom_norm(hp, sub):
                            # 64/denom via ACT exp(-ln(denom/64)); DVE
                            # reciprocal on [1,N] measured ~6.6us
                            head = 2 * hp + sub
                            ops = pv_tiles[sub]
                            rsf = attn_s.tile(
                                [1, N], FP32, tag="rsf", name="rsf"
                            )
                            lns = attn_s.tile(
                                [1, N], FP32, tag="lns", name="lns"
                            )
                            nc.scalar.activation(
                                out=lns, in_=ops[HD : HD + 1, :],
                                func=AF.Ln,
                            )
                            nc.scalar.activation(
                                out=rsf, in_=lns, func=AF.Exp, scale=-1.0,
                            )
                            rd = rsd.tile([1, N], FP32, tag="rd", name="rd")
                            nc.sync.dma_start(out=rd, in_=rsf)
                            rbs = attn_b.tile(
                                [HD, N], FP32, tag="rbs", name="rbs"
                            )
                            bsrc = bass.AP(
                                tensor=rd.tensor,
                                offset=rd.offset,
                                ap=[[0, HD], *rd.ap[1:]],
                            )
                            nc.sync.dma_start(out=rbs, in_=bsrc)
                            r0 = (head % 2) * HD
                            nc.vector.tensor_tensor(
                                out=oT8[r0 : r0 + HD, head // 2, :],
                                in0=ops[0:HD, :], in1=rbs,
                                op=ALU.mult,
                            )

                        def make_v():
                            # v rides pair 0's exp shadow on PE; psum
                            # shares the "ops" ring with qkT/PV
                            for m in range(TOK):
                                ps = psO.tile(
                                    [P, N], FP32, tag="ops", name="ops"
                                )
                                for j in range(CJ):
                                    for n0, n1 in ((0, 512), (512, 768)):
                                        nc.tensor.matmul(
                                            ps[:, n0:n1],
                                            lhsT=hT8[:, 2 * j : 2 * j + 2,
                                                     m * P : (m + 1) * P],
                                            rhs=wv[j][:, :, n0:n1],
                                            start=(j == 0),
                                            stop=(j == CJ - 1),
                                            perf_mode=DR,
                                        )
                                jc, i = m // 2, m % 2
                                nc.vector.tensor_scalar(
                                    out=vx[jc][:, i, :, 0:HD],
                                    in0=ps[:, 0:C].rearrange(
                                        "p (h d) -> p h d", h=H
                                    ),
                                    scalar1=iq, scalar2=None, op0=ALU.mult,
                                )

                        # software pipeline: PV for pair hp-1 is emitted
                        # in four fine chunks between pair hp's early
                        # score rounds so the ACT exp stream never
                        # starves behind a PE burst
                        make_qkT(0)
                        make_qkT(CT)
                        for hp in range(H // 2):
                            for kc in range(TOK):
                                scores_exp(hp, kc)
                                if hp > 0:
                                    if kc == 1:
                                        pv_chunk(hp - 1, 0, 0)
                                    elif kc == 2:
                                        pv_chunk(hp - 1, 0, 1)
                                        denom_norm(hp - 1, 0)
                                    elif kc == 3:
                                        pv_chunk(hp - 1, 1, 0)
                                    elif kc == 4:
                                        pv_chunk(hp - 1, 1, 1)
                                        denom_norm(hp - 1, 1)
                            if hp == 0:
                                make_v()
                            if hp + 1 < H // 2:
                                make_qkT(hp + 1)
                                make_qkT(CT + hp + 1)
                            # proj/MLP weight DMAs ride the attention phase
                            if hp == 1:
                                for j in range(CJ):
                                    wp.append(dr_load(
                                        w_proj, 2 * j * P, 0, C, E5, f"wp{j}"
                                    ))
                                    wf1.append(dr_load(
                                        w_fc1, 2 * j * P, 0, HID, E4,
                                        f"wf1_{j}"
                                    ))
                            if hp == 3:
                                for jc in range(HJ):
                                    wf2.append(dr_load(
                                        w_fc2, 2 * jc * P, 0, C, E4,
                                        f"wf2_{jc}"
                                    ))
                        hp_last = H // 2 - 1
                        pv_chunk(hp_last, 0, 0)
                        pv_chunk(hp_last, 0, 1)
                        denom_norm(hp_last, 0)
                        pv_chunk(hp_last, 1, 0)
                        pv_chunk(hp_last, 1, 1)
                        denom_norm(hp_last, 1)

                      # --- proj + residual (in place into xt) ----------
                      with tc.tile_pool(name="psP", bufs=2, space="PSUM") as psP:
                          for m in range(TOK):
                              ps = psP.tile([P, C], FP32, tag="pps", name="pps")
                              for j in range(CJ):
                                  for n0, n1 in ((0, 512), (512, 768)):
                                      nc.tensor.matmul(
                                          ps[:, n0:n1],
                                          lhsT=oT8[:, 2 * j : 2 * j + 2,
                                                   m * P : (m + 1) * P],
                                          rhs=wp[j][:, :, n0:n1],
                                          start=(j == 0),
                                          stop=(j == CJ - 1),
                                          perf_mode=DR,
                                      )
                              nc.vector.tensor_tensor(
                                  out=xtall[:, m, :], in0=ps,
                                  in1=xtall[:, m, :], op=ALU.add
                              )

        # xt now holds x1 = x (+ b_proj) + attn_out (+ b_fc2 after LN2)

        # --- LN2 -> h2T8; fc1+gelu -> gT8; fc2 -> back + residual ------
        with tc.tile_pool(name="gTpool", bufs=1) as gTpool:
            gT8 = gTpool.tile([P, HIDT, N], E4, tag="gT8", name="gT8")
            with tc.tile_pool(name="h2Tpool", bufs=1) as h2Tpool:
                h2T8 = h2Tpool.tile([P, CT, N], E4, tag="h2T8", name="h2T8")
                ln_to_feature_major(g2c, b2c, h2T8, bf2_b)

                with tc.tile_pool(name="psU", bufs=2, space="PSUM") as psU:
                    for mh in range(HIDT):
                        ps = psU.tile([P, N], FP32, tag="ups", name="ups")
                        for j in range(CJ):
                            for h in range(2):
                                nc.tensor.matmul(
                                    ps[:, h * 512 : (h + 1) * 512],
                                    lhsT=wf1[j][:, :, mh * P : (mh + 1) * P],
                                    rhs=h2T8[:, 2 * j : 2 * j + 2,
                                             h * 512 : (h + 1) * 512],
                                    start=(j == 0),
                                    stop=(j == CJ - 1),
                                    perf_mode=DR,
                                )
                        nc.scalar.activation(
                            out=gT8[:, mh, :], in_=ps, func=AF.Gelu,
                            bias=bf1c[:, mh : mh + 1], scale=ig,
                        )

            # --- fc2 -> y^T; DMA-transpose back + residual -------------
            with (
                tc.tile_pool(name="psY", bufs=1, space="PSUM") as psY,
                tc.tile_pool(name="yTs", bufs=2) as yTs,
                tc.tile_pool(name="trb", bufs=2, space="PSUM") as trb,
            ):
                for pp in range(2):
                    yps = [
                        psY.tile([P, N], FP32, tag=f"yps{m3}", name=f"yps{m3}")
                        for m3 in range(3)
                    ]
                    for jc in range(HJ):
                        for m3 in range(3):
                            c0 = pp * 384 + m3 * P
                            for h in range(2):
                                nc.tensor.matmul(
                                    yps[m3][:, h * 512 : (h + 1) * 512],
                                    lhsT=wf2[jc][:, :, c0 : c0 + P],
                                    rhs=gT8[:, 2 * jc : 2 * jc + 2,
                                            h * 512 : (h + 1) * 512],
                                    start=(jc == 0),
                                    stop=(jc == HJ - 1),
                                    perf_mode=DR,
                                )
                    for m3 in range(3):
                        c = pp * 3 + m3
                        ysb = yTs.tile([P, N], BF16, tag="ysb", name="ysb")
                        nc.vector.tensor_scalar(
                            out=ysb, in0=yps[m3], scalar1=iy, scalar2=None,
                            op0=ALU.mult,
                        )
                        for g4 in range(2):
                            pst = trb.tile(
                                [P, 512], BF16, tag="ytr", name="ytr"
                            )
                            for mm4 in range(4):
                                m = g4 * 4 + mm4
                                nc.tensor.matmul(
                                    pst[:, mm4 * P : (mm4 + 1) * P],
                                    lhsT=ysb[:, m * P : (m + 1) * P],
                                    rhs=identB,
                                    is_transpose=True,
                                    start=(mm4 == 0), stop=(mm4 == 3),
                                    skip_group_check=True,
                                )
                            sl = xtall[:, g4 * 4 : (g4 + 1) * 4,
                                       c * P : (c + 1) * P]
                            nc.vector.tensor_tensor(
                                out=sl,
                                in0=pst.rearrange("p (m n) -> p m n", m=4),
                                in1=sl,
                                op=ALU.add,
                            )
                            if c == CT - 1:
                                for mm4 in range(4):
                                    m = g4 * 4 + mm4
                                    nc.sync.dma_start(
                                        out=out[m * P : (m + 1) * P, :],
                                        in_=xtall[:, m, :],
                                    )

    ctx_lp.__exit__(None, None, None)
    return out


# ---- wait splitting (walrus allows 1 sync wait/instruction) ----

"""Post-pass: this container's walrus rejects >1 sync wait per instruction.

Tile's sem-assignment freely attaches several waits to one instruction.
Peel all but the last wait onto freshly inserted NoOp instructions on the
same engine, placed immediately before the instruction in its block.

Safety: every wait references a strictly earlier vector-clock tick, and
per-engine instruction streams are tick-ordered, so moving a wait from an
instruction to an immediately preceding same-engine NoOp only strengthens
ordering (the engine blocks slightly earlier); it cannot deadlock.
For DMA instructions the wait moves from the descriptor to the issuing
engine, which delays the enqueue until the sem is reached - conservative
but correct for the same reason.
"""


def split_multi_waits(nc, max_waits: int = 1) -> int:
    n_split = 0
    for f in nc.m.functions:
        for bb in f.blocks:
            insts = list(bb.instructions)
            out = []
            for inst in insts:
                si = inst.sync_info
                waits = list(si.on_wait) if si is not None else []
                if len(waits) > max_waits:
                    n_split += 1
                    peel = waits[:-max_waits]
                    si.on_wait = waits[-max_waits:]
                    for i in range(0, len(peel), max_waits):
                        nop = mybir.InstNoOp(
                            name=f"I-waitfix-{n_split}-{i}",
                            engine=inst.engine,
                            ins=[],
                            outs=[],
                            sync_info=mybir.SyncInfo(
                                on_wait=peel[i : i + max_waits], on_update=[]
                            ),
                        )
                        nc.register_instruction(nop)
                        out.append(nop)
                out.append(inst)
            if len(out) != len(insts):
                bb.instructions[:] = out
    return n_split


# ----------------------------------------------------------------------
# SPMD entry point: full inputs in, full outputs out (8-way batch-parallel)
# ----------------------------------------------------------------------

_N_CORES = 8
_SCALES = {}


def _pow2_scale(w):
    a = float(_np.abs(w).max())
    return float(2.0 ** _np.floor(_np.log2(224.0 / a))) if a > 0 else 1.0


def _prep_weights(inputs):
    f32 = lambda k: _np.asarray(inputs[k], dtype=_np.float32)
    w = {}
    wq = f32("w_qkv")
    wf1 = f32("w_fc1")
    wf2 = f32("w_fc2")
    _SCALES["s_qkv"] = _pow2_scale(wq)
    _SCALES["s_fc1"] = _pow2_scale(wf1)
    _SCALES["s_fc2"] = _pow2_scale(wf2)
    w["w_qkv"] = _np.ascontiguousarray(
        (wq * _SCALES["s_qkv"]).astype(_mld.float8_e4m3)
    )
    w["w_fc1"] = _np.ascontiguousarray(
        (wf1 * _SCALES["s_fc1"]).astype(_mld.float8_e4m3)
    )
    w["w_fc2"] = _np.ascontiguousarray(
        (wf2 * _SCALES["s_fc2"]).astype(_mld.float8_e4m3)
    )
    w["w_proj"] = _np.ascontiguousarray(
        (f32("w_proj") / OSC).astype(_mld.float8_e5m2)
    )
    for k in ("ln1_g", "ln2_g", "b_proj", "b_fc1", "b_fc2"):
        w[k] = _np.ascontiguousarray(f32(k))
    for k in ("ln1_b", "ln2_b"):
        w[k] = _np.ascontiguousarray(f32(k) * SA)
    return w


def _build_program(weights):
    import concourse.tile as tile

    nc = bass.Bass("TRN2", target_bir_lowering=False, debug=False,
                   num_devices=_N_CORES)
    with tile.TileContext(nc) as tc:
        build(
            nc, tc,
            _SCALES["s_qkv"], _SCALES["s_fc1"], _SCALES["s_fc2"],
            with_b_proj=bool(_np.any(weights["b_proj"])),
            with_b_fc2=bool(_np.any(weights["b_fc2"])),
        )
    split_multi_waits(nc)
    return nc


def kernel(**inputs):
    from concourse.bass_utils import run_bass_kernel_spmd

    x = _np.ascontiguousarray(_np.asarray(inputs["x"], dtype=_np.float32))
    assert x.shape == (8, N, C), x.shape
    weights = _prep_weights(inputs)
    nc = _build_program(weights)
    in_maps = [{"x": x[b], **weights} for b in range(_N_CORES)]
    res = run_bass_kernel_spmd(nc, in_maps, list(range(_N_CORES)))
    out = _np.stack([res.results[b]["out"] for b in range(_N_CORES)])
    return out.astype(_np.float32)


# revision 53
# speedup vs baseline: 1.0158x; 1.0158x over previous
"""Self-contained Trainium2 kernel for nn_Block (dense transformer block),
8-way batch-parallel across NeuronCores.

fp8 version.  All matmul operands are fp8 (e4m3 except w_proj in e5m2);
contraction-pair packing via MatmulPerfMode.DoubleRow (two K=128 chunks
per instruction) on every weight-stationary matmul and on P@V.  Scores
stay plain fp8 (K=64 contraction can't pair).  fp32 accumulation in
PSUM throughout; residuals, LN stats and softmax denominators fp32.

Scale plumbing (zero extra instructions):
  - LN outputs are scaled x16 (folded into rstd via sqrt((var+eps)/256))
    and ln biases are pre-scaled x16 on the host.
  - w_qkv/w_fc1/w_fc2 are pre-scaled by a power of two (absmax -> ~224)
    on the host; the inverse rides existing evacuation scale slots
    (tensor_scalar mult / gelu input scale).
  - softmax: the ones column in v is 1/64 so the P@V denominator row is
    sum(exp)/64; o^T = pv / bcast(denom/64) = 64*o lands in e4m3 range,
    and w_proj is pre-divided by 64 (e5m2) so proj PSUM is true-scale.

Layouts: token-major LN with bn_stats (x lives in one [128, 8, 768]
tile); PE-transposes to feature-major run as groups of four bf16
[128,128] blocks into one [128,512] PSUM tile (a single accumulation
group with disjoint writes - the bank's lazy zero makes them plain
stores), evacuated by one instruction that applies the LN affine and
the fp8 cast (alternating ACT/DVE).  hT/h2T/oT live as single
[128, 6, 1024] tiles and gT as [128, 24, 1024] so DoubleRow rhs/lhsT
pairs are plain slices.  v is token-major [128, 2, 12, 128] per kc-pair
(64 v dims, a 1/64 column emitting denom/64 in PSUM row 64, padding for
the dual-fp8 ldweights 16B alignment rules); q^T,k^T are feature-major
[128, 1024] per block; exp(S^T) is written by ACT directly to fp8 into
kc-paired, hp-parity double-buffered [128, 2, 1024] tiles; P@V
DoubleRow contracts 256 k-tokens per instruction.  The attention loop
is software-pipelined: PV(hp-1) + its softmax normalization run between
the first two and the remaining six kc score/exp rounds of pair hp, so
ACT (the attention bottleneck: 96 exps of [128,1024]) never starves.
Softmax normalization: ACT exp(-ln(denom/64)) on the [1, N] denominator
row, DMA broadcast via a DRAM bounce, DVE multiply into fp8 oT (x64).
proj/MLP weight DMAs are issued mid-attention to hide their transfers.
The fc2 output transposes back token-major via grouped PE transposes
with the residual added in place; each token's output DMA issues as
soon as its last channel block lands.

Measured (neuron-profile, 8 cores): ~350-370us vs 461us baseline; PE
~230us busy (DoubleRow K=256 at 1 col/cycle = 2x bf16 FLOPs; K=32 DR
sub-tiles measured SLOWER - don't fold scores), ACT ~185us (exp 101us
is the attention floor), DVE ~114us.  Run-to-run clock variance ~10%.
"""

import numpy as _np
import ml_dtypes as _mld

import concourse.bass as bass
import concourse.mybir as mybir
from concourse.masks import make_identity

AF = mybir.ActivationFunctionType
ALU = mybir.AluOpType
DR = mybir.MatmulPerfMode.DoubleRow
FP32 = mybir.dt.float32
BF16 = mybir.dt.bfloat16
E4 = mybir.dt.float8e4
E5 = mybir.dt.float8e5

N, C, H, HD, HID = 1024, 768, 12, 64, 4 * 768
P = 128
TOK = N // P        # 8 token chunks
CT = C // P         # 6 channel chunks
CJ = CT // 2        # 3 channel DoubleRow pairs
HIDT = HID // P     # 24 hidden chunks
HJ = HIDT // 2      # 12 hidden DoubleRow pairs
KCJ = TOK // 2      # 4 k-token DoubleRow pairs
EPS = 1e-5
SCALE = HD ** (-0.5)
SA = 16.0           # LN activation pre-scale (folded into rstd)
OSC = 64.0          # softmax output pre-scale (ones col = 1/OSC)


def build(nc: bass.Bass, tc, s_qkv, s_fc1, s_fc2,
          with_b_proj=True, with_b_fc2=True):
    ctx_lp = nc.allow_low_precision(
        reason="fp8 matmul operands, fp32 accum; validated vs fp32 ref"
    )
    ctx_lp.__enter__()
    x = nc.dram_tensor("x", [N, C], FP32, kind="ExternalInput").ap()
    ln1_g = nc.dram_tensor("ln1_g", [C], FP32, kind="ExternalInput").ap()
    ln1_b = nc.dram_tensor("ln1_b", [C], FP32, kind="ExternalInput").ap()
    w_qkv = nc.dram_tensor("w_qkv", [C, 3 * C], E4, kind="ExternalInput").ap()
    w_proj = nc.dram_tensor("w_proj", [C, C], E5, kind="ExternalInput").ap()
    b_proj = nc.dram_tensor("b_proj", [C], FP32, kind="ExternalInput").ap()
    ln2_g = nc.dram_tensor("ln2_g", [C], FP32, kind="ExternalInput").ap()
    ln2_b = nc.dram_tensor("ln2_b", [C], FP32, kind="ExternalInput").ap()
    w_fc1 = nc.dram_tensor("w_fc1", [C, HID], E4, kind="ExternalInput").ap()
    b_fc1 = nc.dram_tensor("b_fc1", [HID], FP32, kind="ExternalInput").ap()
    w_fc2 = nc.dram_tensor("w_fc2", [HID, C], E4, kind="ExternalInput").ap()
    b_fc2 = nc.dram_tensor("b_fc2", [C], FP32, kind="ExternalInput").ap()
    out = nc.dram_tensor("out", [N, C], FP32, kind="ExternalOutput").ap()

    iq = 1.0 / (SA * s_qkv)   # q/k/v evacuation scale
    ig = 1.0 / (SA * s_fc1)   # gelu input scale
    iy = 1.0 / s_fc2          # fc2 evacuation scale

    with (
        tc.tile_pool(name="singles", bufs=1) as singles,
        tc.tile_pool(name="xpool", bufs=1) as xpool,
        tc.tile_pool(name="wpool", bufs=1) as wpool,
        tc.tile_pool(name="temps", bufs=4) as temps,
        tc.tile_pool(name="stats", bufs=4) as stats,
    ):
        eps_t = singles.tile([P, 1], FP32, tag="eps", name="eps")
        nc.vector.memset(eps_t, EPS / (SA * SA))
        identB = singles.tile([P, P], BF16, tag="identB", name="identB")
        make_identity(nc, identB)

        def col_load(vec_ap, n_ch, tag):
            """[n_ch*128] DRAM vector -> [128, n_ch] SBUF per-partition."""
            t = singles.tile([P, n_ch], FP32, tag=tag, name=tag)
            nc.sync.dma_start(out=t, in_=vec_ap.rearrange("(c p) -> p c", p=P))
            return t

        def bcast_load(vec_ap, tag):
            """[768] DRAM vector -> [128, 768] broadcast across partitions."""
            t = singles.tile([P, C], FP32, tag=tag, name=tag)
            src = bass.AP(
                tensor=vec_ap.tensor,
                offset=vec_ap.offset,
                ap=[[0, P], *vec_ap.ap],
            )
            nc.sync.dma_start(out=t, in_=src)
            return t

        g1c = col_load(ln1_g, CT, "g1c")
        b1c = col_load(ln1_b, CT, "b1c")  # pre-scaled x16 on host
        g2c = col_load(ln2_g, CT, "g2c")
        b2c = col_load(ln2_b, CT, "b2c")  # pre-scaled x16 on host
        bf1c = col_load(b_fc1, HIDT, "bf1c")
        bp_b = bcast_load(b_proj, "bp_b") if with_b_proj else None
        bf2_b = bcast_load(b_fc2, "bf2_b") if with_b_fc2 else None

        # --- weights: DoubleRow-paired SBUF tiles, loaded up front ------
        def dr_load(src2d, r0, c0, m, dtype, tag):
            """rows [r0, r0+256), cols [c0, c0+m) -> [128, 2, m] tile."""
            t = wpool.tile([P, 2, m], dtype, tag=tag, name=tag)
            nc.sync.dma_start(
                out=t,
                in_=src2d[r0 : r0 + 2 * P, c0 : c0 + m].rearrange(
                    "(i p) m -> p i m", i=2
                ),
            )
            return t

        # --- load x first (critical path), then qkv weights; proj/MLP
        # weights are loaded later so their DMAs ride the attention phase
        xtall = xpool.tile([P, TOK, C], FP32, tag="xtall", name="xtall")
        for m in range(TOK):
            nc.sync.dma_start(
                out=xtall[:, m, :], in_=x[m * P : (m + 1) * P, :]
            )
        wq = [dr_load(w_qkv, 2 * j * P, 0, 2 * C, E4, f"wq{j}") for j in range(CJ)]
        wv = [dr_load(w_qkv, 2 * j * P, 2 * C, C, E4, f"wv{j}") for j in range(CJ)]
        wp, wf1, wf2 = [], [], []  # loaded mid-attention (see hp loop)

        def ln_normalize(src_tile):
            """token-major [128, 768] fp32 -> bf16  16*(x-mu)/sigma."""
            st = stats.tile([P, 3, 6], FP32, tag="bnst", name="bnst")
            src3 = src_tile.rearrange("p (s d) -> p s d", s=3)
            for s in range(3):
                nc.vector.bn_stats(out=st[:, s, :], in_=src3[:, s, :])
            mv = stats.tile([P, 2], FP32, tag="bnmv", name="bnmv")
            nc.vector.bn_aggr(out=mv, in_=st)
            rstd = stats.tile([P, 1], FP32, tag="bnrstd", name="bnrstd")
            nc.scalar.activation(
                out=rstd, in_=mv[:, 1:2], func=AF.Sqrt, bias=eps_t,
                scale=1.0 / (SA * SA),
            )
            nc.vector.reciprocal(out=rstd, in_=rstd)  # = 16/sigma
            hn = temps.tile([P, C], BF16, tag="hn", name="hn")
            nc.vector.tensor_scalar(
                out=hn, in0=src_tile, scalar1=mv[:, 0:1], scalar2=rstd,
                op0=ALU.subtract, op1=ALU.mult,
            )
            return hn

        def ln_to_feature_major(gcol, bcol, h8, fold_bias):
            """LN each token tile, PE-transpose in groups of 4 blocks into
            one [128, 512] bf16 PSUM tile (single accumulation group with
            disjoint writes), evacuate with the LN affine + fp8 cast."""
            with tc.tile_pool(name="psT", bufs=1, space="PSUM") as pstp:
                for half in range(2):
                    hns = []
                    for mm4 in range(4):
                        m = half * 4 + mm4
                        hns.append(ln_normalize(xtall[:, m, :]))
                    for c in range(CT):
                        pst = pstp.tile(
                            [P, 512], BF16, tag=f"pst{c}", name=f"pst{c}"
                        )
                        for mm4 in range(4):
                            nc.tensor.matmul(
                                pst[:, mm4 * P : (mm4 + 1) * P],
                                lhsT=hns[mm4][:, c * P : (c + 1) * P],
                                rhs=identB,
                                is_transpose=True,
                                start=(mm4 == 0), stop=(mm4 == 3),
                                skip_group_check=True,
                            )
                        dst = h8[:, c, half * 512 : (half + 1) * 512]
                        if c % 2 == 0:
                            nc.scalar.activation(
                                out=dst, in_=pst, func=AF.Identity,
                                scale=gcol[:, c : c + 1],
                                bias=bcol[:, c : c + 1],
                            )
                        else:
                            nc.vector.tensor_scalar(
                                out=dst, in0=pst,
                                scalar1=gcol[:, c : c + 1],
                                scalar2=bcol[:, c : c + 1],
                                op0=ALU.mult, op1=ALU.add,
                            )
                    if fold_bias is not None:
                        for mm4 in range(4):
                            m = half * 4 + mm4
                            nc.vector.tensor_tensor(
                                out=xtall[:, m, :], in0=xtall[:, m, :],
                                in1=fold_bias, op=ALU.add,
                            )

        with tc.tile_pool(name="hTpool", bufs=1) as hTpool:
            hT8 = hTpool.tile([P, CT, N], E4, tag="hT8", name="hT8")
            ln_to_feature_major(g1c, b1c, hT8, bp_b)

            with tc.tile_pool(name="qkTpool", bufs=1) as qkTpool:
                qkT = [
                    qkTpool.tile([P, N], E4, tag=f"qkT{i}", name=f"qkT{i}")
                    for i in range(2 * CT)
                ]
                with tc.tile_pool(name="vxpool", bufs=1) as vxpool:
                    # per head: 64 v columns, a 1/64 column emitting
                    # denom/64 in PSUM row 64, then padding to 128 —
                    # dual-fp8 ldweights needs 16B-aligned strides/bases
                    vx = [
                        vxpool.tile(
                            [P, 2, H, 2 * HD], E4, tag=f"vx{jc}", name=f"vx{jc}"
                        )
                        for jc in range(KCJ)
                    ]
                    # v ones columns (v itself is produced inside the
                    # attention phase, riding pair 0's exp shadow)
                    for jc in range(KCJ):
                        for i in range(2):
                            nc.vector.memset(
                                vx[jc][:, i, :, HD : 2 * HD], 1.0 / OSC
                            )

                    with tc.tile_pool(name="oTpool", bufs=1) as oTpool:
                      oT8 = oTpool.tile(
                          [P, CT, N], E4, tag="oT8", name="oT8"
                      )
                      with (
                        tc.tile_pool(name="psSc", bufs=2, space="PSUM") as psSc,
                        tc.tile_pool(name="psO", bufs=2, space="PSUM") as psO,
                        tc.tile_pool(name="expp", bufs=1) as expp,
                        tc.tile_pool(name="attn_s", bufs=4) as attn_s,
                        tc.tile_pool(name="attn_b", bufs=3) as attn_b,
                        tc.tile_pool(name="rsd", bufs=4, space="DRAM") as rsd,
                      ):
                        # double-buffered over head pairs (hp parity) so
                        # PV(hp-1) reads while scores(hp) write
                        expS = [
                            {
                                (sub, jc): expp.tile(
                                    [P, 2, N], E4,
                                    tag=f"expS{par}_{sub}_{jc}",
                                    name=f"expS{par}_{sub}_{jc}",
                                )
                                for sub in range(2)
                                for jc in range(KCJ)
                            }
                            for par in range(2)
                        ]

                        def make_qkT(i):
                            ps = psO.tile([P, N], FP32, tag="ops", name="ops")
                            for h in range(2):
                                for j in range(CJ):
                                    nc.tensor.matmul(
                                        ps[:, h * 512 : (h + 1) * 512],
                                        lhsT=wq[j][:, :, i * P : (i + 1) * P],
                                        rhs=hT8[:, 2 * j : 2 * j + 2,
                                                h * 512 : (h + 1) * 512],
                                        start=(j == 0),
                                        stop=(j == CJ - 1),
                                        perf_mode=DR,
                                    )
                            nc.vector.tensor_scalar(
                                out=qkT[i], in0=ps, scalar1=iq, scalar2=None,
                                op0=ALU.mult,
                            )

                        def scores_exp(hp, kc):
                            qt = qkT[hp]
                            kt = qkT[CT + hp]
                            for sub in range(2):
                                sp = psSc.tile(
                                    [P, N], FP32, tag="scps", name="scps"
                                )
                                rows = slice(sub * HD, (sub + 1) * HD)
                                for qh in range(2):
                                    nc.tensor.matmul(
                                        sp[:, qh * 512 : (qh + 1) * 512],
                                        lhsT=kt[rows, kc * P : (kc + 1) * P],
                                        rhs=qt[rows,
                                               qh * 512 : (qh + 1) * 512],
                                        start=True,
                                        stop=True,
                                    )
                                nc.scalar.activation(
                                    out=expS[hp % 2][sub, kc // 2][
                                        :, kc % 2, :
                                    ],
                                    in_=sp, func=AF.Exp, scale=SCALE,
                                )

                        pv_tiles = {}

                        def pv_chunk(hp, sub, qh):
                            # P@V quarter (DoubleRow over kc pairs); the
                            # 1/64 column puts denom/64 in PSUM row 64
                            head = 2 * hp + sub
                            if qh == 0:
                                pv_tiles[sub] = psO.tile(
                                    [P, N], FP32, tag="ops", name="ops"
                                )
                            ops = pv_tiles[sub]
                            qs = slice(qh * 512, (qh + 1) * 512)
                            for jc in range(KCJ):
                                nc.tensor.matmul(
                                    ops[:, qs],
                                    lhsT=vx[jc][:, :, head, :],
                                    rhs=expS[hp % 2][sub, jc][:, :, qs],
                                    start=(jc == 0),
                                    stop=(jc == KCJ - 1),
                                    perf_mode=DR,
                                )

                        def denom_norm(hp, sub):
                            # 64/denom via ACT exp(-ln(denom/64)); DVE
                            # reciprocal on [1,N] measured ~6.6us
                            head = 2 * hp + sub
                            ops = pv_tiles[sub]
                            rsf = attn_s.tile(
                                [1, N], FP32, tag="rsf", name="rsf"
                            )
                            lns = attn_s.tile(
                                [1, N], FP32, tag="lns", name="lns"
                            )
                            nc.scalar.activation(
                                out=lns, in_=ops[HD : HD + 1, :],
                                func=AF.Ln,
                            )
                            nc.scalar.activation(
                                out=rsf, in_=lns, func=AF.Exp, scale=-1.0,
                            )
                            rd = rsd.tile([1, N], FP32, tag="rd", name="rd")
                            nc.sync.dma_start(out=rd, in_=rsf)
                            rbs = attn_b.tile(
                                [HD, N], FP32, tag="rbs", name="rbs"
                            )
                            bsrc = bass.AP(
                                tensor=rd.tensor,
                                offset=rd.offset,
                                ap=[[0, HD], *rd.ap[1:]],
                            )
                            nc.sync.dma_start(out=rbs, in_=bsrc)
                            r0 = (head % 2) * HD
                            nc.vector.tensor_tensor(
                                out=oT8[r0 : r0 + HD, head // 2, :],
                                in0=ops[0:HD, :], in1=rbs,
                                op=ALU.mult,
                            )

                        def make_v():
                            # v rides pair 0's exp shadow on PE; psum
                            # shares the "ops" ring with qkT/PV
                            for m in range(TOK):
                                ps = psO.tile(
                                    [P, N], FP32, tag="ops", name="ops"
                                )
                                for j in range(CJ):
                                    for n0, n1 in ((0, 512), (512, 768)):
                                        nc.tensor.matmul(
                                            ps[:, n0:n1],
                                            lhsT=hT8[:, 2 * j : 2 * j + 2,
                                                     m * P : (m + 1) * P],
                                            rhs=wv[j][:, :, n0:n1],
                                            start=(j == 0),
                                            stop=(j == CJ - 1),
                                            perf_mode=DR,
                                        )
                                jc, i = m // 2, m % 2
                                nc.vector.tensor_scalar(
                                    out=vx[jc][:, i, :, 0:HD],
                                    in0=ps[:, 0:C].rearrange(
                                        "p (h d) -> p h d", h=H
                                    ),
                                    scalar1=iq, scalar2=None, op0=ALU.mult,
                                )

                        # software pipeline: PV for pair hp-1 is emitted
                        # in four fine chunks between pair hp's early
                        # score rounds so the ACT exp stream never
                        # starves behind a PE burst
                        make_qkT(0)
                        make_qkT(CT)
                        for hp in range(H // 2):
                            for kc in range(2):
                                scores_exp(hp, kc)
                            if hp > 0:
                                # compact PV bursts, denom chain right
                                # behind each so the Ln never dams the
                                # exp stream; one score round between
                                # bursts refills ACT's runahead buffer
                                pv_chunk(hp - 1, 0, 0)
                                pv_chunk(hp - 1, 0, 1)
                                denom_norm(hp - 1, 0)
                            scores_exp(hp, 2)
                            if hp > 0:
                                pv_chunk(hp - 1, 1, 0)
                                pv_chunk(hp - 1, 1, 1)
                                denom_norm(hp - 1, 1)
                            for kc in range(3, TOK):
                                scores_exp(hp, kc)
                            if hp == 0:
                                make_v()
                            if hp + 1 < H // 2:
                                make_qkT(hp + 1)
                                make_qkT(CT + hp + 1)
                            # proj/MLP weight DMAs ride the attention phase
                            if hp == 1:
                                for j in range(CJ):
                                    wp.append(dr_load(
                                        w_proj, 2 * j * P, 0, C, E5, f"wp{j}"
                                    ))
                                    wf1.append(dr_load(
                                        w_fc1, 2 * j * P, 0, HID, E4,
                                        f"wf1_{j}"
                                    ))
                            if hp == 3:
                                for jc in range(HJ):
                                    wf2.append(dr_load(
                                        w_fc2, 2 * jc * P, 0, C, E4,
                                        f"wf2_{jc}"
                                    ))
                        hp_last = H // 2 - 1
                        pv_chunk(hp_last, 0, 0)
                        pv_chunk(hp_last, 0, 1)
                        denom_norm(hp_last, 0)
                        pv_chunk(hp_last, 1, 0)
                        pv_chunk(hp_last, 1, 1)
                        denom_norm(hp_last, 1)

                      # --- proj + residual (in place into xt) ----------
                      with tc.tile_pool(name="psP", bufs=2, space="PSUM") as psP:
                          for m in range(TOK):
                              ps = psP.tile([P, C], FP32, tag="pps", name="pps")
                              for j in range(CJ):
                                  for n0, n1 in ((0, 512), (512, 768)):
                                      nc.tensor.matmul(
                                          ps[:, n0:n1],
                                          lhsT=oT8[:, 2 * j : 2 * j + 2,
                                                   m * P : (m + 1) * P],
                                          rhs=wp[j][:, :, n0:n1],
                                          start=(j == 0),
                                          stop=(j == CJ - 1),
                                          perf_mode=DR,
                                      )
                              nc.vector.tensor_tensor(
                                  out=xtall[:, m, :], in0=ps,
                                  in1=xtall[:, m, :], op=ALU.add
                              )

        # xt now holds x1 = x (+ b_proj) + attn_out (+ b_fc2 after LN2)

        # --- LN2 -> h2T8; fc1+gelu -> gT8; fc2 -> back + residual ------
        with tc.tile_pool(name="gTpool", bufs=1) as gTpool:
            gT8 = gTpool.tile([P, HIDT, N], E4, tag="gT8", name="gT8")
            with tc.tile_pool(name="h2Tpool", bufs=1) as h2Tpool:
                h2T8 = h2Tpool.tile([P, CT, N], E4, tag="h2T8", name="h2T8")
                ln_to_feature_major(g2c, b2c, h2T8, bf2_b)

                with tc.tile_pool(name="psU", bufs=2, space="PSUM") as psU:
                    for mh in range(HIDT):
                        ps = psU.tile([P, N], FP32, tag="ups", name="ups")
                        for j in range(CJ):
                            for h in range(2):
                                nc.tensor.matmul(
                                    ps[:, h * 512 : (h + 1) * 512],
                                    lhsT=wf1[j][:, :, mh * P : (mh + 1) * P],
                                    rhs=h2T8[:, 2 * j : 2 * j + 2,
                                             h * 512 : (h + 1) * 512],
                                    start=(j == 0),
                                    stop=(j == CJ - 1),
                                    perf_mode=DR,
                                )
                        nc.scalar.activation(
                            out=gT8[:, mh, :], in_=ps, func=AF.Gelu,
                            bias=bf1c[:, mh : mh + 1], scale=ig,
                        )

            # --- fc2 -> y^T; DMA-transpose back + residual -------------
            with (
                tc.tile_pool(name="psY", bufs=1, space="PSUM") as psY,
                tc.tile_pool(name="yTs", bufs=2) as yTs,
                tc.tile_pool(name="trb", bufs=2, space="PSUM") as trb,
            ):
                for pp in range(2):
                    yps = [
                        psY.tile([P, N], FP32, tag=f"yps{m3}", name=f"yps{m3}")
                        for m3 in range(3)
                    ]
                    for jc in range(HJ):
                        for m3 in range(3):
                            c0 = pp * 384 + m3 * P
                            for h in range(2):
                                nc.tensor.matmul(
                                    yps[m3][:, h * 512 : (h + 1) * 512],
                                    lhsT=wf2[jc][:, :, c0 : c0 + P],
                                    rhs=gT8[:, 2 * jc : 2 * jc + 2,
                                            h * 512 : (h + 1) * 512],
                                    start=(jc == 0),
                                    stop=(jc == HJ - 1),
                                    perf_mode=DR,
                                )
                    for m3 in range(3):
                        c = pp * 3 + m3
                        ysb = yTs.tile([P, N], BF16, tag="ysb", name="ysb")
                        nc.vector.tensor_scalar(
                            out=ysb, in0=yps[m3], scalar1=iy, scalar2=None,
                            op0=ALU.mult,
                        )
                        for g4 in range(2):
                            pst = trb.tile(
                                [P, 512], BF16, tag="ytr", name="ytr"
                            )
                            for mm4 in range(4):
                                m = g4 * 4 + mm4
                                nc.tensor.matmul(
                                    pst[:, mm4 * P : (mm4 + 1) * P],
                                    lhsT=ysb[:, m * P : (m + 1) * P],
                                    rhs=identB,
                                    is_transpose=True,
                                    start=(mm4 == 0), stop=(mm4 == 3),
                                    skip_group_check=True,
                                )
                            sl = xtall[:, g4 * 4 : (g4 + 1) * 4,
                                       c * P : (c + 1) * P]
                            nc.vector.tensor_tensor(
                                out=sl,
                                in0=pst.rearrange("p (m n) -> p m n", m=4),
                                in1=sl,
                                op=ALU.add,
                            )
                            if c == CT - 1:
                                for mm4 in range(4):
                                    m = g4 * 4 + mm4
                                    nc.sync.dma_start(
                                        out=out[m * P : (m + 1) * P, :],
                                        in_=xtall[:, m, :],
                                    )

    ctx_lp.__exit__(None, None, None)
    return out


# ---- wait splitting (walrus allows 1 sync wait/instruction) ----

"""Post-pass: this container's walrus rejects >1 sync wait per instruction.

Tile's sem-assignment freely attaches several waits to one instruction.
Peel all but the last wait onto freshly inserted NoOp instructions on the
same engine, placed immediately before the instruction in its block.

Safety: every wait references a strictly earlier vector-clock tick, and
per-engine instruction streams are tick-ordered, so moving a wait from an
instruction to an immediately preceding same-engine NoOp only strengthens
ordering (the engine blocks slightly earlier); it cannot deadlock.
For DMA instructions the wait moves from the descriptor to the issuing
engine, which delays the enqueue until the sem is reached - conservative
but correct for the same reason.
"""


def split_multi_waits(nc, max_waits: int = 1) -> int:
    n_split = 0
    for f in nc.m.functions:
        for bb in f.blocks:
            insts = list(bb.instructions)
            out = []
            for inst in insts:
                si = inst.sync_info
                waits = list(si.on_wait) if si is not None else []
                if len(waits) > max_waits:
                    n_split += 1
                    peel = waits[:-max_waits]
                    si.on_wait = waits[-max_waits:]
                    for i in range(0, len(peel), max_waits):
                        nop = mybir.InstNoOp(
                            name=f"I-waitfix-{n_split}-{i}",
                            engine=inst.engine,
                            ins=[],
                            outs=[],
                            sync_info=mybir.SyncInfo(
                                on_wait=peel[i : i + max_waits], on_update=[]
                            ),
                        )
                        nc.register_instruction(nop)
                        out.append(nop)
                out.append(inst)
            if len(out) != len(insts):
                bb.instructions[:] = out
    return n_split


# ----------------------------------------------------------------------
# SPMD entry point: full inputs in, full outputs out (8-way batch-parallel)
# ----------------------------------------------------------------------

_N_CORES = 8
_SCALES = {}


def _pow2_scale(w):
    a = float(_np.abs(w).max())
    return float(2.0 ** _np.floor(_np.log2(224.0 / a))) if a > 0 else 1.0


def _prep_weights(inputs):
    f32 = lambda k: _np.asarray(inputs[k], dtype=_np.float32)
    w = {}
    wq = f32("w_qkv")
    wf1 = f32("w_fc1")
    wf2 = f32("w_fc2")
    _SCALES["s_qkv"] = _pow2_scale(wq)
    _SCALES["s_fc1"] = _pow2_scale(wf1)
    _SCALES["s_fc2"] = _pow2_scale(wf2)
    w["w_qkv"] = _np.ascontiguousarray(
        (wq * _SCALES["s_qkv"]).astype(_mld.float8_e4m3)
    )
    w["w_fc1"] = _np.ascontiguousarray(
        (wf1 * _SCALES["s_fc1"]).astype(_mld.float8_e4m3)
    )
    w["w_fc2"] = _np.ascontiguousarray(
        (wf2 * _SCALES["s_fc2"]).astype(_mld.float8_e4m3)
    )
    w["w_proj"] = _np.ascontiguousarray(
        (f32("w_proj") / OSC).astype(_mld.float8_e5m2)
    )
    for k in ("ln1_g", "ln2_g", "b_proj", "b_fc1", "b_fc2"):
        w[k] = _np.ascontiguousarray(f32(k))
    for k in ("ln1_b", "ln2_b"):
        w[k] = _np.ascontiguousarray(f32(k) * SA)
    return w


def _build_program(weights):
    import concourse.tile as tile

    nc = bass.Bass("TRN2", target_bir_lowering=False, debug=False,
                   num_devices=_N_CORES)
    with tile.TileContext(nc) as tc:
        build(
            nc, tc,
            _SCALES["s_qkv"], _SCALES["s_fc1"], _SCALES["s_fc2"],
            with_b_proj=bool(_np.any(weights["b_proj"])),
            with_b_fc2=bool(_np.any(weights["b_fc2"])),
        )
    split_multi_waits(nc)
    return nc


def kernel(**inputs):
    from concourse.bass_utils import run_bass_kernel_spmd

    x = _np.ascontiguousarray(_np.asarray(inputs["x"], dtype=_np.float32))
    assert x.shape == (8, N, C), x.shape
    weights = _prep_weights(inputs)
    nc = _build_program(weights)
    in_maps = [{"x": x[b], **weights} for b in range(_N_CORES)]
    res = run_bass_kernel_spmd(nc, in_maps, list(range(_N_CORES)))
    out = _np.stack([res.results[b]["out"] for b in range(_N_CORES)])
    return out.astype(_np.float32)
